# revision 13
# baseline (speedup 1.0000x reference)
"""LocalFrameAttentionWithDiffuser on 8 TRN2 NeuronCores.

Sharding: head-parallel. Each core computes 2 of the 16 heads end-to-end
(QKV projection for its 128 hd-dims, chunked local attention, partial
output projection Y_c = O_c @ Wo[c-slice]); the host sums the 8 partial
Y tensors in fp32 and adds the bias once.

Shapes (hardcoded from the problem):
  x [1,16,256,1024] -> tokens T=4096, D=1024, H=16 heads, HD=64,
  chunks C=4 of L=1024 tokens; chunk i attends to chunks {i-1, i}
  (chunk 0 only to itself).

Device layout notes:
  - all matmul operands are bf16 (x, weights converted on host; DMA and
    SBUF halve vs fp32; PE rate is the same as fp32r).
  - S^T = K^T.T @ Q^T per (chunk, head) with ctx on partitions; exp runs
    on the scalar engine over [128, 2, 512] PSUM pairs (two ctx tiles
    per instruction to amortize the fixed access latency) and the
    softmax sum comes from a ones-column appended to V in the AV matmul.
  - chunk 0's missing previous chunk is handled by not issuing those ctx
    tiles (exactly reproduces the -inf mask).
  - partial Y is written bf16 straight after the output projection (the
    bias is added host-side, once); the host accumulates in fp32.
  - projections of chunk c+1 are interleaved between the attention heads
    of chunk c so the scalar engine (the bottleneck) never starves.
"""

import os
from contextlib import ExitStack

import numpy as np
import ml_dtypes

import concourse.bass as bass
import concourse.tile as tile
from concourse import bacc, mybir
from concourse.bass_utils import run_bass_kernel_spmd

F32 = mybir.dt.float32
BF16 = mybir.dt.bfloat16

B, F, N, D = 1, 16, 256, 1024
H, HD = 16, 64
CS = 4
C = F // CS            # 4 chunks
L = CS * N             # 1024 tokens per chunk
T = F * N              # 4096 tokens
NCORES = 8
HPC = H // NCORES      # 2 heads per core
HDB = HPC * HD         # 128 hd dims per core
SCALE = 1.0 / np.sqrt(HD)

TOK_TILE = 512
NDT = D // 128         # 8 contraction tiles for projections
NCT = T // 128         # 32 ctx tiles of 128


def build_kernel(nc, tc, outs, ins, ctx):
    xt = [ins[f"xt{i}"] for i in range(NDT)]
    wq, wk, wv, wo, ident = ins["wq"], ins["wk"], ins["wv"], ins["wo"], ins["ident"]
    y = outs["y"]

    wpool = ctx.enter_context(tc.tile_pool(name="weights", bufs=1))
    act_pool = ctx.enter_context(tc.tile_pool(name="acts", bufs=1))
    vs_pool = ctx.enter_context(tc.tile_pool(name="vstage", bufs=2))
    a_pool = ctx.enter_context(tc.tile_pool(name="attn", bufs=4))
    sum_pool = ctx.enter_context(tc.tile_pool(name="sums", bufs=4))
    bc_pool = ctx.enter_context(tc.tile_pool(name="bcast", bufs=4))
    yout_pool = ctx.enter_context(tc.tile_pool(name="yout", bufs=4))
    ps_pool = ctx.enter_context(tc.tile_pool(name="ps", bufs=1, space="PSUM"))

    # ---- persistent weights / activations ----
    wq_sb = wpool.tile([128, NDT, HDB], BF16, tag="wq")
    wk_sb = wpool.tile([128, NDT, HDB], BF16, tag="wk")
    wv_sb = wpool.tile([128, NDT, HDB], BF16, tag="wv")
    wo_sb = wpool.tile([128, D], BF16, tag="wo")
    id_sb = wpool.tile([128, 128], BF16, tag="id")
    xt_sb = [wpool.tile([128, T], BF16, name=f"xs{i}") for i in range(NDT)]

    qt_sb = act_pool.tile([128, T], BF16, tag="qt")   # Q^T (2 heads stacked)
    kt_sb = act_pool.tile([128, T], BF16, tag="kt")   # K^T
    ot_sb = act_pool.tile([128, T], BF16, tag="ot")   # normalized O^T
    # V per head: [128 ctx, ct, 64 hd + ones]
    v_sb = [act_pool.tile([128, NCT, HD + 1], BF16, name=f"v{h}")
            for h in range(HPC)]
    for h in range(HPC):
        nc.vector.memset(v_sb[h][:, :, HD:HD + 1], 1.0)

    def dma_x(c):
        for i in range(NDT):
            nc.sync.dma_start(xt_sb[i][:, c * L:(c + 1) * L],
                              xt[i][:, c * L:(c + 1) * L])

    dma_x(0)
    nc.sync.dma_start(wq_sb[:], wq.rearrange("(i p) m -> p i m", i=NDT))
    nc.sync.dma_start(wk_sb[:], wk.rearrange("(i p) m -> p i m", i=NDT))
    nc.sync.dma_start(wv_sb[:], wv.rearrange("(i p) m -> p i m", i=NDT))
    nc.sync.dma_start(wo_sb[:], wo[:, :])
    nc.sync.dma_start(id_sb[:], ident[:, :])

    def proj_qk(j, w_sb, dst, tag):
        tok = slice(j * TOK_TILE, (j + 1) * TOK_TILE)
        p_ps = ps_pool.tile([128, TOK_TILE], F32, tag=tag, name=f"p{j}_{tag}")
        for i in range(NDT):
            nc.tensor.matmul(p_ps[:], w_sb[:, i, :], xt_sb[i][:, tok],
                             start=(i == 0), stop=(i == NDT - 1))
        nc.vector.tensor_copy(dst[:, tok], p_ps[:])

    def proj_v(j, tag):
        tok = slice(j * TOK_TILE, (j + 1) * TOK_TILE)
        v_ps = ps_pool.tile([128, TOK_TILE], F32, tag=tag, name=f"vps{j}")
        for i in range(NDT):
            nc.tensor.matmul(v_ps[:], wv_sb[:, i, :], xt_sb[i][:, tok],
                             start=(i == 0), stop=(i == NDT - 1))
        vt_stage = vs_pool.tile([128, TOK_TILE], BF16, tag="vs", name=f"vst{j}")
        nc.vector.tensor_copy(vt_stage[:], v_ps[:])
        for m in range(4):
            ct = j * 4 + m
            vtr = ps_pool.tile([128, 128], BF16, tag="yv", name=f"vtr{j}_{m}")
            nc.tensor.transpose(vtr[:], vt_stage[:, m * 128:(m + 1) * 128],
                                id_sb[:])
            for h in range(HPC):
                nc.vector.tensor_copy(v_sb[h][:, ct, 0:HD],
                                      vtr[:, h * HD:(h + 1) * HD])

    def attn_head(c, th, h, tag):
        """Attention for (chunk c, token half th, head h) -> ot_sb."""
        tok = slice(c * L + th * TOK_TILE, c * L + (th + 1) * TOK_TILE)
        hr = slice(h * HD, (h + 1) * HD)
        ct0 = max(0, 8 * (c - 1))
        cts = list(range(ct0, 8 * (c + 1)))
        o_ps = ps_pool.tile([HD + 1, TOK_TILE], F32, tag=f"o{tag}",
                            name=f"ops{c}_{th}_{h}")
        npair = len(cts) // 2
        for gi in range(npair):
            s2 = ps_pool.tile([128, 2, TOK_TILE], F32, tag="s", bufs=2,
                              name=f"sps{c}_{th}_{h}_{gi}")
            for kk in range(2):
                ct = cts[2 * gi + kk]
                nc.tensor.matmul(s2[:, kk, :],
                                 kt_sb[hr, ct * 128:(ct + 1) * 128],
                                 qt_sb[hr, tok], start=True, stop=True)
            a_t = a_pool.tile([128, 2, TOK_TILE], BF16, tag="a",
                              name=f"a{c}_{th}_{h}_{gi}")
            nc.scalar.activation(a_t[:], s2[:],
                                 mybir.ActivationFunctionType.Exp,
                                 scale=float(SCALE))
            for kk in range(2):
                ct = cts[2 * gi + kk]
                nc.tensor.matmul(o_ps[:], v_sb[h][:, ct, :], a_t[:, kk, :],
                                 start=(gi == 0 and kk == 0),
                                 stop=(gi == npair - 1 and kk == 1))
        s_sum = sum_pool.tile([1, TOK_TILE], F32, tag="ss", name=f"ssum{c}_{th}_{h}")
        nc.vector.reciprocal(s_sum[:], o_ps[HD:HD + 1, :])
        r_bc = bc_pool.tile([HD, TOK_TILE], F32, tag="bc", name=f"bc{c}_{th}_{h}")
        nc.gpsimd.partition_broadcast(r_bc[:], s_sum[0:1, :])
        nc.vector.tensor_mul(ot_sb[hr, tok], o_ps[0:HD, :], r_bc[:])

    def yproj(c, th):
        """Output projection for the 4 token tiles of half-chunk (c, th)."""
        m0 = c * 8 + th * 4
        for dh in range(2):
            ds = slice(dh * TOK_TILE, (dh + 1) * TOK_TILE)
            y_sb = yout_pool.tile([128, 4, TOK_TILE], BF16, tag="yo",
                                  name=f"yo{c}_{th}_{dh}")
            for mi in range(4):
                m = m0 + mi
                y_ps = ps_pool.tile([128, TOK_TILE], F32, tag="yv",
                                    name=f"yps{m}_{dh}")
                nc.tensor.matmul(y_ps[:], ot_sb[:, m * 128:(m + 1) * 128],
                                 wo_sb[:, ds], start=True, stop=True)
                nc.vector.tensor_copy(y_sb[:, mi, :], y_ps[:])
            nc.sync.dma_start(
                y[m0 * 128:(m0 + 4) * 128, ds]
                .rearrange("(m p) t -> p m t", m=4), y_sb[:])

    # ---- schedule ----
    # prologue: chunk-0 projections, pipelined through 3 psum slots
    for j, tg in ((0, "qk"), (1, "o0")):
        proj_qk(j, wq_sb, qt_sb, tg)
    for j, tg in ((0, "o1"), (1, "qk")):
        proj_qk(j, wk_sb, kt_sb, tg)
    proj_v(0, "o0")
    proj_v(1, "o1")
    for c in range(C):
        if c + 1 < C:
            dma_x(c + 1)
        j0, j1 = 2 * (c + 1), 2 * (c + 1) + 1
        attn_head(c, 0, 0, 0)
        if c + 1 < C:
            proj_qk(j0, wq_sb, qt_sb, "qk")
        attn_head(c, 0, 1, 1)
        if c + 1 < C:
            proj_qk(j1, wq_sb, qt_sb, "qk")
        if c < C - 1:
            yproj(c, 0)
        if c + 1 < C:
            proj_qk(j0, wk_sb, kt_sb, "qk")
        attn_head(c, 1, 0, 0)
        if c + 1 < C:
            proj_qk(j1, wk_sb, kt_sb, "qk")
        if c == C - 1:
            yproj(c, 0)
        attn_head(c, 1, 1, 1)
        if c + 1 < C:
            proj_v(j0, "qk")
        yproj(c, 1)
        if c + 1 < C:
            proj_v(j1, "qk")


_CACHE = {}


def _build():
    if "nc" in _CACHE:
        return _CACHE["nc"]
    nc = bacc.Bacc(
        "TRN2",
        target_bir_lowering=False,
        debug=False,
        enable_asserts=False,
        num_devices=NCORES,
    )
    ins = {}
    for i in range(NDT):
        ins[f"xt{i}"] = nc.dram_tensor(f"xt{i}", [128, T], BF16,
                                       kind="ExternalInput").ap()
    for nm in ("wq", "wk", "wv"):
        ins[nm] = nc.dram_tensor(nm, [D, HDB], BF16, kind="ExternalInput").ap()
    ins["wo"] = nc.dram_tensor("wo", [HDB, D], BF16, kind="ExternalInput").ap()
    ins["ident"] = nc.dram_tensor("ident", [128, 128], BF16,
                                  kind="ExternalInput").ap()
    outs = {"y": nc.dram_tensor("y", [T, D], BF16, kind="ExternalOutput").ap()}
    with tile.TileContext(nc, trace_sim=False) as tc:
        with ExitStack() as kctx:
            build_kernel(nc, tc, outs, ins, kctx)
    nc.compile()
    _CACHE["nc"] = nc
    return nc


def make_in_maps(x, Wq, Wk, Wv, Wo, bo):
    xv = np.asarray(x, np.float32).reshape(T, D).T          # [D, T]
    x16 = np.ascontiguousarray(xv).astype(ml_dtypes.bfloat16)
    ident = np.eye(128, dtype=np.float32).astype(ml_dtypes.bfloat16)
    in_maps = []
    for core in range(NCORES):
        hs = slice(core * HDB, (core + 1) * HDB)
        m = {f"xt{i}": x16[i * 128:(i + 1) * 128] for i in range(NDT)}
        m["wq"] = np.ascontiguousarray(
            np.asarray(Wq, np.float32)[:, hs]).astype(ml_dtypes.bfloat16)
        m["wk"] = np.ascontiguousarray(
            np.asarray(Wk, np.float32)[:, hs]).astype(ml_dtypes.bfloat16)
        m["wv"] = np.ascontiguousarray(
            np.asarray(Wv, np.float32)[:, hs]).astype(ml_dtypes.bfloat16)
        m["wo"] = np.ascontiguousarray(
            np.asarray(Wo, np.float32)[hs, :]).astype(ml_dtypes.bfloat16)
        m["ident"] = ident
        in_maps.append(m)
    return in_maps


def kernel(x, Wq, Wk, Wv, Wo, bo, _trace=False, _tmpdir=None):
    nc = _build()
    in_maps = make_in_maps(x, Wq, Wk, Wv, Wo, bo)
    res = run_bass_kernel_spmd(
        nc, in_maps, core_ids=list(range(NCORES)),
        trace=_trace, tmpdir=_tmpdir,
        **({"trace_cores": list(range(NCORES))} if _trace else {}),
    )
    if _trace:
        kernel.last_results = res
    y = np.zeros((T, D), dtype=np.float32)
    for r in res.results:
        y += np.asarray(r["y"], dtype=np.float32)
    y += np.asarray(bo, np.float32).reshape(1, D)
    return y.reshape(B, F, N, D)


# revision 14
# speedup vs baseline: 1.0023x; 1.0023x over previous
"""LocalFrameAttentionWithDiffuser on 8 TRN2 NeuronCores.

Sharding: head-parallel. Each core computes 2 of the 16 heads end-to-end
(QKV projection for its 128 hd-dims, chunked local attention, partial
output projection Y_c = O_c @ Wo[c-slice]); the host sums the 8 partial
Y tensors in fp32 and adds the bias once.

Shapes (hardcoded from the problem):
  x [1,16,256,1024] -> tokens T=4096, D=1024, H=16 heads, HD=64,
  chunks C=4 of L=1024 tokens; chunk i attends to chunks {i-1, i}
  (chunk 0 only to itself).

Device layout notes:
  - all matmul operands are bf16 (x, weights converted on host; DMA and
    SBUF halve vs fp32; PE rate is the same as fp32r).
  - S^T = K^T.T @ Q^T per (chunk, head) with ctx on partitions; exp runs
    on the scalar engine over [128, 2, 512] PSUM pairs (two ctx tiles
    per instruction to amortize the fixed access latency) and the
    softmax sum comes from a ones-column appended to V in the AV matmul.
  - chunk 0's missing previous chunk is handled by not issuing those ctx
    tiles (exactly reproduces the -inf mask).
  - partial Y is written bf16 straight after the output projection (the
    bias is added host-side, once); the host accumulates in fp32.
  - projections of chunk c+1 are interleaved between the attention heads
    of chunk c so the scalar engine (the bottleneck) never starves.
"""

import os
from contextlib import ExitStack

import numpy as np
import ml_dtypes

import concourse.bass as bass
import concourse.tile as tile
from concourse import bacc, mybir
from concourse.bass_utils import run_bass_kernel_spmd

F32 = mybir.dt.float32
BF16 = mybir.dt.bfloat16
F8 = mybir.dt.float8e4
DR = mybir.MatmulPerfMode.DoubleRow
WS = 64.0

B, F, N, D = 1, 16, 256, 1024
H, HD = 16, 64
CS = 4
C = F // CS            # 4 chunks
L = CS * N             # 1024 tokens per chunk
T = F * N              # 4096 tokens
NCORES = 8
HPC = H // NCORES      # 2 heads per core
HDB = HPC * HD         # 128 hd dims per core
SCALE = 1.0 / np.sqrt(HD)

TOK_TILE = 512
NDT = D // 128         # 8 contraction tiles for projections
NCT = T // 128         # 32 ctx tiles of 128


def build_kernel(nc, tc, outs, ins, ctx):
    xt = [ins[f"xt{i}"] for i in range(NDT)]
    x8 = [ins[f"x8{i}"] for i in range(NDT)]
    wq, wk8 = ins["wq"], ins["wk8"]
    wv, wo, ident = ins["wv"], ins["wo"], ins["ident"]
    y = outs["y"]

    wpool = ctx.enter_context(tc.tile_pool(name="weights", bufs=1))
    act_pool = ctx.enter_context(tc.tile_pool(name="acts", bufs=1))
    vs_pool = ctx.enter_context(tc.tile_pool(name="vstage", bufs=2))
    a_pool = ctx.enter_context(tc.tile_pool(name="attn", bufs=4))
    sum_pool = ctx.enter_context(tc.tile_pool(name="sums", bufs=4))
    bc_pool = ctx.enter_context(tc.tile_pool(name="bcast", bufs=4))
    yout_pool = ctx.enter_context(tc.tile_pool(name="yout", bufs=2))
    ps_pool = ctx.enter_context(tc.tile_pool(name="ps", bufs=1, space="PSUM"))

    # ---- persistent weights / activations ----
    wq_sb = wpool.tile([128, NDT, HDB], BF16, tag="wq")
    wk_sb = wpool.tile([128, NDT // 2, 2, HDB], F8, tag="wk")
    x8_sb = [wpool.tile([128, 2, T], F8, name=f"x8s{i}") for i in range(NDT)]
    wv_sb = wpool.tile([128, NDT, HDB], BF16, tag="wv")
    wo_sb = wpool.tile([128, D], BF16, tag="wo")
    id_sb = wpool.tile([128, 128], BF16, tag="id")
    xt_sb = [wpool.tile([128, T], BF16, name=f"xs{i}") for i in range(NDT)]

    qt_sb = act_pool.tile([128, T], BF16, tag="qt")   # Q^T (2 heads stacked)
    kt_sb = act_pool.tile([128, T], BF16, tag="kt")   # K^T
    ot_sb = act_pool.tile([128, T], BF16, tag="ot")   # normalized O^T
    # V per head: [128 ctx, ct, 64 hd + ones]
    v_sb = [act_pool.tile([128, NCT, HD + 1], BF16, name=f"v{h}")
            for h in range(HPC)]
    for h in range(HPC):
        nc.vector.memset(v_sb[h][:, :, HD:HD + 1], 1.0)

    def dma_x(c):
        for i in range(NDT):
            nc.sync.dma_start(
                x8_sb[i][:, :, c * L:(c + 1) * L],
                x8[i].rearrange("p (k t) -> p k t", k=2)[:, :, c * L:(c + 1) * L])
        for i in range(NDT):
            nc.sync.dma_start(xt_sb[i][:, c * L:(c + 1) * L],
                              xt[i][:, c * L:(c + 1) * L])

    dma_x(0)
    nc.sync.dma_start(wq_sb[:], wq.rearrange("(i p) m -> p i m", i=NDT))
    nc.sync.dma_start(wk_sb[:],
                      wk8.rearrange("(i p) (k m) -> p i k m", i=NDT // 2, k=2))
    nc.sync.dma_start(wv_sb[:], wv.rearrange("(i p) m -> p i m", i=NDT))
    nc.sync.dma_start(wo_sb[:], wo[:, :])
    nc.sync.dma_start(id_sb[:], ident[:, :])

    def proj_qk(j, w_sb, dst, tag):
        tok = slice(j * TOK_TILE, (j + 1) * TOK_TILE)
        p_ps = ps_pool.tile([128, TOK_TILE], F32, tag=tag, name=f"p{j}_{tag}")
        for i in range(NDT):
            nc.tensor.matmul(p_ps[:], w_sb[:, i, :], xt_sb[i][:, tok],
                             start=(i == 0), stop=(i == NDT - 1))
        nc.vector.tensor_copy(dst[:, tok], p_ps[:])

    def proj_k8(j, tag):
        tok = slice(j * TOK_TILE, (j + 1) * TOK_TILE)
        p_ps = ps_pool.tile([128, TOK_TILE], F32, tag=tag, name=f"k8{j}_{tag}")
        for i in range(NDT):
            nc.tensor.matmul(p_ps[:], wk_sb[:, i % 4, :, :],
                             x8_sb[i][:, :, tok],
                             start=(i == 0), stop=(i == NDT - 1), perf_mode=DR)
        nc.vector.tensor_copy(kt_sb[:, tok], p_ps[:])

    def proj_v(j, tag):
        tok = slice(j * TOK_TILE, (j + 1) * TOK_TILE)
        v_ps = ps_pool.tile([128, TOK_TILE], F32, tag=tag, name=f"vps{j}")
        for i in range(NDT):
            nc.tensor.matmul(v_ps[:], wv_sb[:, i, :], xt_sb[i][:, tok],
                             start=(i == 0), stop=(i == NDT - 1))
        vt_stage = vs_pool.tile([128, TOK_TILE], BF16, tag="vs", name=f"vst{j}")
        nc.vector.tensor_copy(vt_stage[:], v_ps[:])
        for m in range(4):
            ct = j * 4 + m
            vtr = ps_pool.tile([128, 128], BF16, tag="yv", name=f"vtr{j}_{m}")
            nc.tensor.transpose(vtr[:], vt_stage[:, m * 128:(m + 1) * 128],
                                id_sb[:])
            for h in range(HPC):
                nc.vector.tensor_copy(v_sb[h][:, ct, 0:HD],
                                      vtr[:, h * HD:(h + 1) * HD])

    def attn_head(c, th, h, tag):
        """Attention for (chunk c, token half th, head h) -> ot_sb."""
        tok = slice(c * L + th * TOK_TILE, c * L + (th + 1) * TOK_TILE)
        hr = slice(h * HD, (h + 1) * HD)
        ct0 = max(0, 8 * (c - 1))
        cts = list(range(ct0, 8 * (c + 1)))
        o_ps = ps_pool.tile([HD + 1, TOK_TILE], F32, tag=f"o{tag}",
                            name=f"ops{c}_{th}_{h}")
        npair = len(cts) // 2
        for gi in range(npair):
            s2 = ps_pool.tile([128, 2, TOK_TILE], F32, tag="s", bufs=2,
                              name=f"sps{c}_{th}_{h}_{gi}")
            for kk in range(2):
                ct = cts[2 * gi + kk]
                nc.tensor.matmul(s2[:, kk, :],
                                 kt_sb[hr, ct * 128:(ct + 1) * 128],
                                 qt_sb[hr, tok], start=True, stop=True)
            a_t = a_pool.tile([128, 2, TOK_TILE], BF16, tag="a",
                              name=f"a{c}_{th}_{h}_{gi}")
            nc.scalar.activation(a_t[:], s2[:],
                                 mybir.ActivationFunctionType.Exp,
                                 scale=float(SCALE / WS))
            for kk in range(2):
                ct = cts[2 * gi + kk]
                nc.tensor.matmul(o_ps[:], v_sb[h][:, ct, :], a_t[:, kk, :],
                                 start=(gi == 0 and kk == 0),
                                 stop=(gi == npair - 1 and kk == 1))
        s_sum = sum_pool.tile([1, TOK_TILE], F32, tag="ss", name=f"ssum{c}_{th}_{h}")
        nc.vector.reciprocal(s_sum[:], o_ps[HD:HD + 1, :])
        r_bc = bc_pool.tile([HD, TOK_TILE], F32, tag="bc", name=f"bc{c}_{th}_{h}")
        nc.gpsimd.partition_broadcast(r_bc[:], s_sum[0:1, :])
        nc.vector.tensor_mul(ot_sb[hr, tok], o_ps[0:HD, :], r_bc[:])

    def yproj(c, th):
        """Output projection for the 4 token tiles of half-chunk (c, th)."""
        m0 = c * 8 + th * 4
        for dh in range(2):
            ds = slice(dh * TOK_TILE, (dh + 1) * TOK_TILE)
            y_sb = yout_pool.tile([128, 4, TOK_TILE], BF16, tag="yo",
                                  name=f"yo{c}_{th}_{dh}")
            for mi in range(4):
                m = m0 + mi
                y_ps = ps_pool.tile([128, TOK_TILE], F32, tag="yv",
                                    name=f"yps{m}_{dh}")
                nc.tensor.matmul(y_ps[:], ot_sb[:, m * 128:(m + 1) * 128],
                                 wo_sb[:, ds], start=True, stop=True)
                nc.vector.tensor_copy(y_sb[:, mi, :], y_ps[:])
            nc.sync.dma_start(
                y[m0 * 128:(m0 + 4) * 128, ds]
                .rearrange("(m p) t -> p m t", m=4), y_sb[:])

    # ---- schedule ----
    # prologue: chunk-0 projections, pipelined through 3 psum slots
    for j, tg in ((0, "qk"), (1, "o0")):
        proj_qk(j, wq_sb, qt_sb, tg)
    for j, tg in ((0, "o1"), (1, "qk")):
        proj_k8(j, tg)
    proj_v(0, "o0")
    proj_v(1, "o1")
    for c in range(C):
        if c + 1 < C:
            dma_x(c + 1)
        j0, j1 = 2 * (c + 1), 2 * (c + 1) + 1
        attn_head(c, 0, 0, 0)
        if c + 1 < C:
            proj_qk(j0, wq_sb, qt_sb, "qk")
        attn_head(c, 0, 1, 1)
        if c + 1 < C:
            proj_qk(j1, wq_sb, qt_sb, "qk")
        if c < C - 1:
            yproj(c, 0)
        if c + 1 < C:
            proj_k8(j0, "qk")
        attn_head(c, 1, 0, 0)
        if c + 1 < C:
            proj_k8(j1, "qk")
        if c == C - 1:
            yproj(c, 0)
        attn_head(c, 1, 1, 1)
        if c + 1 < C:
            proj_v(j0, "qk")
        yproj(c, 1)
        if c + 1 < C:
            proj_v(j1, "qk")


_CACHE = {}


def _build():
    if "nc" in _CACHE:
        return _CACHE["nc"]
    nc = bacc.Bacc(
        "TRN2",
        target_bir_lowering=False,
        debug=False,
        enable_asserts=False,
        num_devices=NCORES,
    )
    ins = {}
    for i in range(NDT):
        ins[f"xt{i}"] = nc.dram_tensor(f"xt{i}", [128, T], BF16,
                                       kind="ExternalInput").ap()
    for nm in ("wq", "wv"):
        ins[nm] = nc.dram_tensor(nm, [D, HDB], BF16, kind="ExternalInput").ap()
    for i in range(NDT):
        ins[f"x8{i}"] = nc.dram_tensor(f"x8{i}", [128, 2 * T], F8,
                                       kind="ExternalInput").ap()
    ins["wk8"] = nc.dram_tensor("wk8", [D // 2, 2 * HDB], F8,
                                kind="ExternalInput").ap()
    ins["wo"] = nc.dram_tensor("wo", [HDB, D], BF16, kind="ExternalInput").ap()
    ins["ident"] = nc.dram_tensor("ident", [128, 128], BF16,
                                  kind="ExternalInput").ap()
    outs = {"y": nc.dram_tensor("y", [T, D], BF16, kind="ExternalOutput").ap()}
    with tile.TileContext(nc, trace_sim=False) as tc:
        with ExitStack() as kctx:
            build_kernel(nc, tc, outs, ins, kctx)
    nc.compile()
    _CACHE["nc"] = nc
    return nc


def make_in_maps(x, Wq, Wk, Wv, Wo, bo):
    xv = np.asarray(x, np.float32).reshape(T, D).T          # [D, T]
    x16 = np.ascontiguousarray(xv).astype(ml_dtypes.bfloat16)
    ident = np.eye(128, dtype=np.float32).astype(ml_dtypes.bfloat16)
    xhi = xv.astype(ml_dtypes.float8_e4m3)
    xlo = (xv - xhi.astype(np.float32)).astype(ml_dtypes.float8_e4m3)
    x8t = np.concatenate([
        np.ascontiguousarray(
            xx.astype(np.float32).reshape(NDT // 2, 2, 128, T)
            .transpose(0, 2, 1, 3).reshape(NDT // 2, 128, 2 * T))
        .astype(ml_dtypes.float8_e4m3)
        for xx in (xhi, xlo)], axis=0)

    def w8prep(W, hs):
        w = np.asarray(W, np.float32)[:, hs] * WS
        w = w.reshape(NDT // 2, 2, 128, HDB).transpose(0, 2, 1, 3)
        return np.ascontiguousarray(
            w.reshape(D // 2, 2 * HDB)).astype(ml_dtypes.float8_e4m3)

    in_maps = []
    for core in range(NCORES):
        hs = slice(core * HDB, (core + 1) * HDB)
        m = {f"xt{i}": x16[i * 128:(i + 1) * 128] for i in range(NDT)}
        m["wq"] = np.ascontiguousarray(
            np.asarray(Wq, np.float32)[:, hs]).astype(ml_dtypes.bfloat16)
        for i in range(NDT):
            m[f"x8{i}"] = x8t[i]
        m["wk8"] = w8prep(Wk, hs)
        m["wv"] = np.ascontiguousarray(
            np.asarray(Wv, np.float32)[:, hs]).astype(ml_dtypes.bfloat16)
        m["wo"] = np.ascontiguousarray(
            np.asarray(Wo, np.float32)[hs, :]).astype(ml_dtypes.bfloat16)
        m["ident"] = ident
        in_maps.append(m)
    return in_maps


def kernel(x, Wq, Wk, Wv, Wo, bo, _trace=False, _tmpdir=None):
    nc = _build()
    in_maps = make_in_maps(x, Wq, Wk, Wv, Wo, bo)
    res = run_bass_kernel_spmd(
        nc, in_maps, core_ids=list(range(NCORES)),
        trace=_trace, tmpdir=_tmpdir,
        **({"trace_cores": list(range(NCORES))} if _trace else {}),
    )
    if _trace:
        kernel.last_results = res
    y = np.zeros((T, D), dtype=np.float32)
    for r in res.results:
        y += np.asarray(r["y"], dtype=np.float32)
    y += np.asarray(bo, np.float32).reshape(1, D)
    return y.reshape(B, F, N, D)
